# revision 15
# baseline (speedup 1.0000x reference)
"""Trainium2 Bass kernel for the GCN model (nn_GCNModel_57853209477141).

Model: 3x GCNConv(128->128, sym-norm with self loops) with ReLU, question
embedding MLP, concat, 2-layer MLP head -> [50000, 32].

Strategy (8 NeuronCores, single SPMD launch):
- dst-node sharding: global tiles of 128 nodes; snake-deal tiles (sorted by
  edge count) across cores -> 49 slots/core; one compile-time schedule is
  shared by all 8 cores (SPMD = one program).
- GCN norm factorization: agg[v] = dinv[v] * (sum_{e->v} h~[src_e] + h~[v]),
  table rows h~ = dinv * (h @ W), so no per-edge norm is needed.
- aggregation: per slot, per 128-edge chunk one INDIRECT1D gather fetches the
  edge source rows (bf16, one row per dest partition); a 0/1 one-hot
  [edge, dst] built once per slot by iota-compare on DVE is matmul-accumulated
  against the gathered rows in PSUM (bf16 operands, f32 accumulate).
- bias and question terms enter PSUM as rank-1 / one-hot matmuls:
  b term = (1/dinv)[dst] (x) b[f]; qq term = ohg[g,node]^T @ qq[g,:].
  Epilogue is then a single fused scalar op: relu(dinv * psum).
- tables are block-major; the AllGather is split into 7 row-blocks fired
  from the production epilogues so collective traffic overlaps aggregation.

Host preprocessing is index work only (sharding, edge sort, degree counts,
one-hot of graph ids); all O(E*F) / O(N*F*F) float work runs on device.
"""
import os
import sys
import types
from contextlib import ExitStack

import numpy as np

# ---------------------------------------------------------------- constants
N = 50000
E = 800000
G = 64
P = 128
NCORES = 8
TPC = 49  # tile slots per core
SLOT_ROWS = TPC * P  # 6272
NT = NCORES * SLOT_ROWS  # 50176
QD = 768
OUTC = 32
NBLK = 7  # AllGather split blocks
SPB = TPC // NBLK  # slots per block

AGG_DT = os.environ.get("GCN_AGG_DT", "bf16")  # f32 | f32r | bf16
BF16 = AGG_DT == "bf16"


def _install_axon_prof():
    """Register NTFF profile hook if the image's antenv lacks it; neuter
    bucket upload (zero-egress). Harmless when running without tracing."""
    try:
        from antenv import axon_hooks  # noqa: F401
    except ImportError:
        try:
            import antenv
            from trn_agent_boot.trn_boot import _ntff_profile_via_ctypes

            hook = _ntff_profile_via_ctypes("/opt/axon/libaxon_pjrt.so")
            mod = types.ModuleType("antenv.axon_hooks")
            mod.get_axon_ntff_profile_hook = lambda: hook
            mod.set_axon_ntff_profile_hook = lambda h: None
            sys.modules["antenv.axon_hooks"] = mod
            antenv.axon_hooks = mod
        except Exception:
            pass
    try:
        import concourse.bass_utils as bu

        bu.upload_artifacts = lambda tmpdir: "local://" + str(tmpdir)
    except Exception:
        pass


# ---------------------------------------------------------------- host prep
def preprocess(edge_index, batch):
    src = np.asarray(edge_index[0], dtype=np.int64)
    dst = np.asarray(edge_index[1], dtype=np.int64)
    # degree includes self-loops (GCN norm); the self-loop term itself is
    # added on-device from the resident h~ slice, not gathered.
    deg = (np.bincount(dst, minlength=N) + 1).astype(np.float64)
    dinv = (1.0 / np.sqrt(deg)).astype(np.float32)
    rdinv = np.sqrt(deg).astype(np.float32)

    n_tiles = (N + P - 1) // P  # 391
    tile_of_node = np.arange(N) // P

    dst_tile = dst // P
    tile_counts = np.bincount(dst_tile, minlength=n_tiles)

    # snake-deal tiles (sorted by edge count desc) across cores so every
    # slot's per-core counts are nearly equal -> minimal chunk padding
    order_all = np.argsort(-tile_counts, kind="stable")
    core_tiles = [[] for _ in range(NCORES)]
    for r in range(TPC):
        batch_t = order_all[r * NCORES : (r + 1) * NCORES]
        seq = range(NCORES) if r % 2 == 0 else range(NCORES - 1, -1, -1)
        for j, c in enumerate(seq):
            core_tiles[c].append(int(batch_t[j]) if j < len(batch_t) else -1)
    core_of_tile = np.full(n_tiles, -1, dtype=np.int64)
    slot_of_tile = np.full(n_tiles, -1, dtype=np.int64)
    for c in range(NCORES):
        for s, t in enumerate(core_tiles[c]):
            if t >= 0:
                core_of_tile[t] = c
                slot_of_tile[t] = s

    # unified block-major table layout (all 3 layers)
    blk = slot_of_tile[tile_of_node] // SPB
    table_row = (
        blk * (NCORES * SPB * P)
        + core_of_tile[tile_of_node] * (SPB * P)
        + (slot_of_tile[tile_of_node] % SPB) * P
        + (np.arange(N) % P)
    )

    order = np.argsort(dst_tile, kind="stable")
    src_sorted = src[order]
    dst_sorted = dst[order]
    sorted_tiles = dst_tile[order]
    tile_starts = np.searchsorted(sorted_tiles, np.arange(n_tiles))
    tile_ends = np.searchsorted(sorted_tiles, np.arange(n_tiles), side="right")

    cnts = np.zeros((NCORES, TPC), dtype=np.int64)
    for c in range(NCORES):
        for s in range(TPC):
            t = core_tiles[c][s]
            if t >= 0:
                cnts[c, s] = tile_ends[t] - tile_starts[t]
    chunks = np.maximum(1, -(-cnts.max(axis=0) // P))  # per-slot max over cores
    chunk_base = np.cumsum(np.concatenate([[0], chunks[:-1]])).astype(int)
    TCH = int(chunks.sum())

    idx32 = np.zeros((NCORES, P, TCH), dtype=np.int32)
    dstin_T = np.full((NCORES, P, TCH), -1.0, dtype=np.float32)
    for c in range(NCORES):
        for s in range(TPC):
            t = core_tiles[c][s]
            if t < 0:
                continue
            lo, hi = tile_starts[t], tile_ends[t]
            rows = table_row[src_sorted[lo:hi]]
            din = dst_sorted[lo:hi] % P
            nch = int(chunks[s])
            L = nch * P
            a = np.zeros(L, dtype=np.int32)
            a[: len(rows)] = rows
            d = np.full(L, -1.0, dtype=np.float32)
            d[: len(din)] = din
            cb = chunk_base[s]
            idx32[c, :, cb : cb + nch] = a.reshape(nch, P).T
            dstin_T[c, :, cb : cb + nch] = d.reshape(nch, P).T

    dinv_slot = np.zeros((NCORES, P, TPC), dtype=np.float32)
    rdinv_row = np.zeros((NCORES, 1, SLOT_ROWS), dtype=np.float32)
    gidvec = np.full((NCORES, SLOT_ROWS), -1, dtype=np.int64)
    node_perm = np.full((NCORES, SLOT_ROWS), -1, dtype=np.int64)
    batch = np.asarray(batch, dtype=np.int64)
    for c in range(NCORES):
        for s in range(TPC):
            t = core_tiles[c][s]
            if t < 0:
                continue
            v0 = t * P
            v1 = min(v0 + P, N)
            n = v1 - v0
            dinv_slot[c, :n, s] = dinv[v0:v1]
            rdinv_row[c, 0, s * P : s * P + n] = rdinv[v0:v1]
            gidvec[c, s * P : s * P + n] = batch[v0:v1]
            node_perm[c, s * P : s * P + n] = np.arange(v0, v1)

    ohg = (gidvec[:, None, :] == np.arange(G)[None, :, None]).astype(np.float32)

    return dict(
        chunks=chunks,
        TCH=TCH,
        idx32=idx32,
        dstin_T=dstin_T,
        dinv_slot=dinv_slot,
        rdinv_row=rdinv_row,
        ohg=ohg,
        node_perm=node_perm,
    )


# ------------------------------------------------------------- bass program
def build_program(chunks):
    import concourse.bacc as bacc
    import concourse.bass as bass
    import concourse.tile as tile
    from concourse import mybir
    from concourse.masks import make_identity

    FDT = {
        "f32": mybir.dt.float32,
        "f32r": mybir.dt.float32r,
        "bf16": mybir.dt.bfloat16,
    }[AGG_DT]
    F32 = mybir.dt.float32
    I32 = mybir.dt.int32

    chunks = np.asarray(chunks)
    chunk_base = np.cumsum(np.concatenate([[0], chunks[:-1]])).astype(int)
    TCH = int(chunks.sum())

    nc = bacc.Bacc("TRN2", target_bir_lowering=False)
    dp = nc.declare_dram_parameter
    xT = dp("xT", [P, SLOT_ROWS], FDT, isOutput=False)
    idx_in = dp("idx32", [P, TCH], I32, isOutput=False)
    dstin = dp("dstin", [P, TCH], FDT, isOutput=False)
    iota_in = dp("iota_in", [P, P], FDT, isOutput=False)
    dinv_in = dp("dinv_in", [P, TPC], F32, isOutput=False)
    rdinv_in = dp("rdinv_in", [1, SLOT_ROWS], F32, isOutput=False)
    brow_in = dp("brow_in", [1, 3 * P], F32, isOutput=False)
    ones_in = dp("ones_in", [1, P], F32, isOutput=False)
    W_in = [dp(f"W{i}", [P, P], FDT, isOutput=False) for i in range(3)]
    fc1a_in = dp("fc1a", [P, P], FDT, isOutput=False)
    fc2w_in = dp("fc2w", [P, OUTC], FDT, isOutput=False)
    fc2brow_in = dp("fc2brow", [1, OUTC], F32, isOutput=False)
    ohg_in = dp("ohg", [G, SLOT_ROWS], FDT, isOutput=False)
    qeT_in = dp("qeT", [QD, G], F32, isOutput=False)
    fc0w_in = dp("fc0w", [QD, P], F32, isOutput=False)
    fc0bb_in = dp("fc0bb", [P, P], F32, isOutput=False)
    fc1b_in = dp("fc1b", [P, P], F32, isOutput=False)
    fc1bb_in = dp("fc1bb", [P, P], F32, isOutput=False)
    out_d = dp("out", [SLOT_ROWS, OUTC], F32, isOutput=True)

    cc_in = nc.dram_tensor("cc_in", [SLOT_ROWS, P], FDT)
    tables = [
        nc.dram_tensor(f"table{l}", [NT, P], FDT, addr_space="Shared")
        for l in range(3)
    ]

    with tile.TileContext(nc) as tc, ExitStack() as ctx:
        const = ctx.enter_context(tc.tile_pool(name="const", bufs=1))
        gp = ctx.enter_context(tc.tile_pool(name="gp", bufs=24))
        ohp = ctx.enter_context(tc.tile_pool(name="ohp", bufs=3))
        psp = ctx.enter_context(tc.tile_pool(name="psp", bufs=3, space="PSUM"))
        psagg = ctx.enter_context(tc.tile_pool(name="psagg", bufs=3, space="PSUM"))
        pst = ctx.enter_context(tc.tile_pool(name="pst", bufs=2, space="PSUM"))
        epi = ctx.enter_context(tc.tile_pool(name="epi", bufs=3))

        # ---- constants (xT + W0 first: they gate layer-0 production,
        # which gates the startup AllGathers)
        xT_sb = const.tile([P, SLOT_ROWS], FDT)
        nc.sync.dma_start(out=xT_sb[:], in_=xT[:])
        W_sb = []
        for i in range(3):
            w = const.tile([P, P], FDT, tag=f"W{i}")
            nc.sync.dma_start(out=w[:], in_=W_in[i][:])
            W_sb.append(w)
        dinv_sb = const.tile([P, TPC], F32)
        nc.sync.dma_start(out=dinv_sb[:], in_=dinv_in[:])
        iota_sb = const.tile([P, P], FDT)
        nc.sync.dma_start(out=iota_sb[:], in_=iota_in[:])
        idx_sb = const.tile([P, TCH], I32)
        nc.scalar.dma_start(out=idx_sb[:], in_=idx_in[:])
        dstin_sb = const.tile([P, TCH], FDT)
        nc.scalar.dma_start(out=dstin_sb[:], in_=dstin[:])
        rdinv_sb = const.tile([1, SLOT_ROWS], F32)
        nc.sync.dma_start(out=rdinv_sb[:], in_=rdinv_in[:])
        brow_sb = const.tile([1, 3 * P], F32)
        nc.sync.dma_start(out=brow_sb[:], in_=brow_in[:])
        ones_sb = const.tile([1, P], F32)
        nc.sync.dma_start(out=ones_sb[:], in_=ones_in[:])
        fc1a_sb = const.tile([P, P], FDT)
        nc.sync.dma_start(out=fc1a_sb[:], in_=fc1a_in[:])
        fc2w_sb = const.tile([P, OUTC], FDT)
        nc.sync.dma_start(out=fc2w_sb[:], in_=fc2w_in[:])
        fc2brow_sb = const.tile([1, OUTC], F32)
        nc.sync.dma_start(out=fc2brow_sb[:], in_=fc2brow_in[:])
        ohg_sb = const.tile([G, SLOT_ROWS], FDT)
        nc.sync.dma_start(out=ohg_sb[:], in_=ohg_in[:])
        ident = const.tile([P, P], F32)
        make_identity(nc, ident[:])
        ident_r = const.tile([P, P], FDT, tag="ident_r")
        nc.vector.tensor_copy(out=ident_r[:], in_=ident[:])

        # ---- question path: qq = relu(qe@fc0+fc0_b)@fc1b + fc1_b (on-chip)
        qe_sb = const.tile([P, 6 * G], F32)
        fc0w_sb = const.tile([P, 6 * P], F32)
        for k in range(6):
            nc.sync.dma_start(
                out=qe_sb[:, k * G : (k + 1) * G], in_=qeT_in[k * P : (k + 1) * P, :]
            )
            nc.sync.dma_start(
                out=fc0w_sb[:, k * P : (k + 1) * P],
                in_=fc0w_in[k * P : (k + 1) * P, :],
            )
        fc0bb_sb = const.tile([P, P], F32)
        nc.sync.dma_start(out=fc0bb_sb[:], in_=fc0bb_in[:])
        fc1b_sb = const.tile([P, P], F32)
        nc.sync.dma_start(out=fc1b_sb[:], in_=fc1b_in[:])
        fc1bb_sb = const.tile([P, P], F32)
        nc.sync.dma_start(out=fc1bb_sb[:], in_=fc1bb_in[:])

        pq = psp.tile([G, P], F32, space="PSUM", tag="mm")
        for k in range(6):
            nc.tensor.matmul(
                out=pq[:],
                lhsT=qe_sb[:, k * G : (k + 1) * G],
                rhs=fc0w_sb[:, k * P : (k + 1) * P],
                start=(k == 0),
                stop=(k == 5),
            )
        qtmp = epi.tile([G, P], F32, tag="qtmp")
        nc.vector.tensor_tensor(
            out=qtmp[:], in0=pq[:], in1=fc0bb_sb[:G, :], op=mybir.AluOpType.add
        )
        qrelu = epi.tile([G, P], F32, tag="qrelu")
        nc.scalar.activation(
            out=qrelu[:], in_=qtmp[:], func=mybir.ActivationFunctionType.Relu
        )
        pqt = psp.tile([P, G], F32, space="PSUM", tag="mm")
        nc.tensor.transpose(out=pqt[:], in_=qrelu[:], identity=ident[:G, :G])
        qT = epi.tile([P, G], F32, tag="qT")
        nc.scalar.copy(out=qT[:], in_=pqt[:])
        pqq = psp.tile([G, P], F32, space="PSUM", tag="mm")
        nc.tensor.matmul(
            out=pqq[:], lhsT=qT[:], rhs=fc1b_sb[:], start=True, stop=True
        )
        qtmp2 = epi.tile([G, P], F32, tag="qtmp2")
        nc.vector.tensor_tensor(
            out=qtmp2[:], in0=pqq[:], in1=fc1bb_sb[:G, :], op=mybir.AluOpType.add
        )
        qq_bf = const.tile([G, P], FDT, tag="qq_bf")
        nc.vector.tensor_copy(out=qq_bf[:], in_=qtmp2[:])

        # resident own-slice h~ buffers (self-loop term source), layer parity
        hs_keep = [
            const.tile([P, SLOT_ROWS], FDT, tag=f"hsk{i}", name=f"hsk{i}")
            for i in range(2)
        ]

        def allgather_block(l, j):
            r0 = j * SPB * P
            r1 = (j + 1) * SPB * P
            nc.gpsimd.collective_compute(
                "AllGather",
                mybir.AluOpType.bypass,
                replica_groups=[list(range(NCORES))],
                ins=[cc_in[r0:r1].opt()],
                outs=[
                    tables[l][
                        j * NCORES * SPB * P : (j + 1) * NCORES * SPB * P
                    ].opt()
                ],
            )

        # ---- layer 0 production: h~0 = dinv * (x @ W0), block AGs
        for s in range(TPC):
            pp = psp.tile([P, P], F32, space="PSUM", tag="mm")
            nc.tensor.matmul(
                out=pp[:],
                lhsT=xT_sb[:, s * P : (s + 1) * P],
                rhs=W_sb[0][:],
                start=True,
                stop=True,
            )
            hs = hs_keep[0][:, s * P : (s + 1) * P]
            nc.scalar.activation(
                out=hs,
                in_=pp[:],
                func=mybir.ActivationFunctionType.Copy,
                scale=dinv_sb[:, s : s + 1],
            )
            nc.sync.dma_start(out=cc_in[s * P : (s + 1) * P, :], in_=hs)
            if (s + 1) % SPB == 0:
                allgather_block(0, s // SPB)

        # ---- 3 aggregation layers
        for l in range(3):
            table = tables[l]
            for s in range(TPC):
                nch = int(chunks[s])
                cb = int(chunk_base[s])
                ps = psagg.tile([P, P], F32, space="PSUM", tag="agg")
                # one-hot for the whole slot in one DVE op
                oh = ohp.tile([P, nch * P], FDT, tag="oh")
                nc.vector.tensor_tensor(
                    out=oh[:].rearrange("p (k j) -> p k j", k=nch),
                    in0=dstin_sb[:, cb : cb + nch].to_broadcast([P, nch, P]),
                    in1=iota_sb[:]
                    .rearrange("p (one j) -> p one j", one=1)
                    .to_broadcast([P, nch, P]),
                    op=mybir.AluOpType.is_equal,
                )
                for k in range(nch):
                    g = gp.tile([P, P], FDT, tag="g")
                    nc.gpsimd.indirect_dma_start(
                        out=g[:],
                        out_offset=None,
                        in_=table[:],
                        in_offset=bass.IndirectOffsetOnAxis(
                            ap=idx_sb[:, cb + k : cb + k + 1], axis=0
                        ),
                    )
                    nc.tensor.matmul(
                        out=ps[:],
                        lhsT=oh[:, k * P : (k + 1) * P],
                        rhs=g[:],
                        start=(k == 0),
                        stop=False,
                    )
                # self-loop term: ps += I @ hs_keep[l%2][:, slot]
                nc.tensor.matmul(
                    out=ps[:],
                    lhsT=ident_r[:],
                    rhs=hs_keep[l % 2][:, s * P : (s + 1) * P],
                    start=False,
                    stop=False,
                )
                # bias term: ps += (1/dinv)[dst] (x) b[f]  (rank-1)
                nc.tensor.matmul(
                    out=ps[:],
                    lhsT=rdinv_sb[0:1, s * P : (s + 1) * P],
                    rhs=brow_sb[0:1, l * P : (l + 1) * P],
                    start=False,
                    stop=True,
                )
                # fused epilogue: h = relu(dinv*psum)
                hrelu = epi.tile([P, P], FDT, tag="hrelu")
                nc.scalar.activation(
                    out=hrelu[:],
                    in_=ps[:],
                    func=mybir.ActivationFunctionType.Relu,
                    scale=dinv_sb[:, s : s + 1],
                )
                pt = pst.tile([P, P], FDT, space="PSUM", tag="pt")
                nc.tensor.transpose(out=pt[:], in_=hrelu[:], identity=ident_r[:])
                hT = epi.tile([P, P], FDT, tag="hT")
                nc.vector.tensor_copy(out=hT[:], in_=pt[:])
                if l < 2:
                    # produce next layer h~ and stage for allgather
                    pp2 = psp.tile([P, P], F32, space="PSUM", tag="mm")
                    nc.tensor.matmul(
                        out=pp2[:],
                        lhsT=hT[:],
                        rhs=W_sb[l + 1][:],
                        start=True,
                        stop=True,
                    )
                    hs2 = hs_keep[(l + 1) % 2][:, s * P : (s + 1) * P]
                    nc.scalar.activation(
                        out=hs2,
                        in_=pp2[:],
                        func=mybir.ActivationFunctionType.Copy,
                        scale=dinv_sb[:, s : s + 1],
                    )
                    nc.sync.dma_start(
                        out=cc_in[s * P : (s + 1) * P, :], in_=hs2
                    )
                    if (s + 1) % SPB == 0:
                        allgather_block(l + 1, s // SPB)
                else:
                    # MLP head: u = relu(h3@fc1a + qq[gid]); out = u@fc2+b
                    pm = psp.tile([P, P], F32, space="PSUM", tag="mm")
                    nc.tensor.matmul(
                        out=pm[:], lhsT=hT[:], rhs=fc1a_sb[:], start=True,
                        stop=False,
                    )
                    nc.tensor.matmul(
                        out=pm[:],
                        lhsT=ohg_sb[:, s * P : (s + 1) * P],
                        rhs=qq_bf[:],
                        start=False,
                        stop=True,
                    )
                    ur = epi.tile([P, P], FDT, tag="ur")
                    nc.scalar.activation(
                        out=ur[:], in_=pm[:],
                        func=mybir.ActivationFunctionType.Relu,
                    )
                    pt2 = pst.tile([P, P], FDT, space="PSUM", tag="pt")
                    nc.tensor.transpose(
                        out=pt2[:], in_=ur[:], identity=ident_r[:]
                    )
                    uT = epi.tile([P, P], FDT, tag="uT")
                    nc.vector.tensor_copy(out=uT[:], in_=pt2[:])
                    po = psp.tile([P, OUTC], F32, space="PSUM", tag="mm")
                    nc.tensor.matmul(
                        out=po[:], lhsT=uT[:], rhs=fc2w_sb[:], start=True,
                        stop=False,
                    )
                    nc.tensor.matmul(
                        out=po[:],
                        lhsT=ones_sb[0:1, :],
                        rhs=fc2brow_sb[0:1, :],
                        start=False,
                        stop=True,
                    )
                    ob = epi.tile([P, OUTC], F32, tag="ob")
                    nc.vector.tensor_copy(out=ob[:], in_=po[:])
                    nc.sync.dma_start(
                        out=out_d[s * P : (s + 1) * P, :], in_=ob[:]
                    )
    nc.compile()
    return nc


# ---------------------------------------------------------------- interface
_CACHE = {}


def kernel(**inputs):
    trace = bool(int(os.environ.get("GCN_TRACE", "0")))
    if trace:
        _install_axon_prof()
    from concourse.bass_utils import run_bass_kernel_spmd

    x = np.ascontiguousarray(np.asarray(inputs["x"], dtype=np.float32))
    qe = np.asarray(inputs["question_embedding"], dtype=np.float32)
    pp = preprocess(inputs["edge_index"], inputs["batch"])
    chunks = pp["chunks"]

    key = tuple(chunks.tolist())
    if key not in _CACHE:
        _CACHE[key] = build_program(chunks)
    nc = _CACHE[key]

    fdt = np.dtype("bfloat16") if BF16 else np.float32
    W = [np.asarray(inputs[f"W{i}"], np.float32) for i in range(3)]
    b = [np.asarray(inputs[f"b{i}"], np.float32) for i in range(3)]
    fc0_w = np.asarray(inputs["fc0_w"], np.float32)
    fc0_b = np.asarray(inputs["fc0_b"], np.float32)
    fc1_w = np.asarray(inputs["fc1_w"], np.float32)
    fc1_b = np.asarray(inputs["fc1_b"], np.float32)
    fc2_w = np.asarray(inputs["fc2_w"], np.float32)
    fc2_b = np.asarray(inputs["fc2_b"], np.float32)

    iota = np.broadcast_to(np.arange(P, dtype=np.float32), (P, P)).astype(fdt)
    common = {
        "iota_in": np.ascontiguousarray(iota),
        "W0": W[0].astype(fdt),
        "W1": W[1].astype(fdt),
        "W2": W[2].astype(fdt),
        "brow_in": np.concatenate(b).reshape(1, 3 * P).astype(np.float32),
        "ones_in": np.ones((1, P), np.float32),
        "qeT": np.ascontiguousarray(qe.T),
        "fc0w": fc0_w,
        "fc0bb": np.broadcast_to(fc0_b, (P, P)).copy(),
        "fc1a": np.ascontiguousarray(fc1_w[:P]).astype(fdt),
        "fc1b": np.ascontiguousarray(fc1_w[P:]),
        "fc1bb": np.broadcast_to(fc1_b, (P, P)).copy(),
        "fc2w": fc2_w.astype(fdt),
        "fc2brow": fc2_b.reshape(1, OUTC).astype(np.float32),
    }

    in_maps = []
    for c in range(NCORES):
        xTc = np.zeros((P, SLOT_ROWS), dtype=np.float32)
        valid = pp["node_perm"][c] >= 0
        xTc[:, valid] = x[pp["node_perm"][c][valid]].T
        m = dict(common)
        m["xT"] = np.ascontiguousarray(xTc.astype(fdt))
        m["idx32"] = np.ascontiguousarray(pp["idx32"][c])
        m["dstin"] = np.ascontiguousarray(pp["dstin_T"][c].astype(fdt))
        m["dinv_in"] = np.ascontiguousarray(pp["dinv_slot"][c])
        m["rdinv_in"] = np.ascontiguousarray(pp["rdinv_row"][c])
        m["ohg"] = np.ascontiguousarray(pp["ohg"][c].astype(fdt))
        in_maps.append(m)

    res = run_bass_kernel_spmd(
        nc,
        in_maps,
        list(range(NCORES)),
        trace=trace,
    )
    kernel.last_result = res

    out = np.zeros((N, OUTC), dtype=np.float32)
    for c in range(NCORES):
        valid = pp["node_perm"][c] >= 0
        out[pp["node_perm"][c][valid]] = res.results[c]["out"][valid]
    return out


# revision 20
# speedup vs baseline: 1.0091x; 1.0091x over previous
"""Trainium2 Bass kernel for the GCN model (nn_GCNModel_57853209477141).

Model: 3x GCNConv(128->128, sym-norm with self loops) with ReLU, question
embedding MLP, concat, 2-layer MLP head -> [50000, 32].

Strategy (8 NeuronCores, single SPMD launch):
- dst-node sharding: global tiles of 128 nodes; snake-deal tiles (sorted by
  edge count) across cores -> 49 slots/core; one compile-time schedule is
  shared by all 8 cores (SPMD = one program).
- GCN norm factorization: agg[v] = dinv[v] * (sum_{e->v} h~[src_e] + h~[v]),
  table rows h~ = dinv * (h @ W), so no per-edge norm is needed.
- aggregation: per slot, per 128-edge chunk one INDIRECT1D gather fetches the
  edge source rows (bf16, one row per dest partition); a 0/1 one-hot
  [edge, dst] built once per slot by iota-compare on DVE is matmul-accumulated
  against the gathered rows in PSUM (bf16 operands, f32 accumulate).
- bias and question terms enter PSUM as rank-1 / one-hot matmuls:
  b term = (1/dinv)[dst] (x) b[f]; qq term = ohg[g,node]^T @ qq[g,:].
  Epilogue is then a single fused scalar op: relu(dinv * psum).
- tables are block-major; the AllGather is split into 7 row-blocks fired
  from the production epilogues so collective traffic overlaps aggregation.

Host preprocessing is index work only (sharding, edge sort, degree counts,
one-hot of graph ids); all O(E*F) / O(N*F*F) float work runs on device.
"""
import os
import sys
import types
from contextlib import ExitStack

import numpy as np

# ---------------------------------------------------------------- constants
N = 50000
E = 800000
G = 64
P = 128
NCORES = 8
TPC = 49  # tile slots per core
SLOT_ROWS = TPC * P  # 6272
NT = NCORES * SLOT_ROWS  # 50176
QD = 768
OUTC = 32
NBLK = 7  # AllGather split blocks
SPB = TPC // NBLK  # slots per block

AGG_DT = os.environ.get("GCN_AGG_DT", "bf16")  # f32 | f32r | bf16
BF16 = AGG_DT == "bf16"


def _install_axon_prof():
    """Register NTFF profile hook if the image's antenv lacks it; neuter
    bucket upload (zero-egress). Harmless when running without tracing."""
    try:
        from antenv import axon_hooks  # noqa: F401
    except ImportError:
        try:
            import antenv
            from trn_agent_boot.trn_boot import _ntff_profile_via_ctypes

            hook = _ntff_profile_via_ctypes("/opt/axon/libaxon_pjrt.so")
            mod = types.ModuleType("antenv.axon_hooks")
            mod.get_axon_ntff_profile_hook = lambda: hook
            mod.set_axon_ntff_profile_hook = lambda h: None
            sys.modules["antenv.axon_hooks"] = mod
            antenv.axon_hooks = mod
        except Exception:
            pass
    try:
        import concourse.bass_utils as bu

        bu.upload_artifacts = lambda tmpdir: "local://" + str(tmpdir)
    except Exception:
        pass


# ---------------------------------------------------------------- host prep
def preprocess(edge_index, batch):
    src = np.asarray(edge_index[0], dtype=np.int64)
    dst = np.asarray(edge_index[1], dtype=np.int64)
    # degree includes self-loops (GCN norm); the self-loop term itself is
    # added on-device from the resident h~ slice, not gathered.
    deg = (np.bincount(dst, minlength=N) + 1).astype(np.float64)
    dinv = (1.0 / np.sqrt(deg)).astype(np.float32)
    rdinv = np.sqrt(deg).astype(np.float32)

    n_tiles = (N + P - 1) // P  # 391
    tile_of_node = np.arange(N) // P

    dst_tile = dst // P
    tile_counts = np.bincount(dst_tile, minlength=n_tiles)

    # snake-deal tiles (sorted by edge count desc) across cores so every
    # slot's per-core counts are nearly equal -> minimal chunk padding
    order_all = np.argsort(-tile_counts, kind="stable")
    core_tiles = [[] for _ in range(NCORES)]
    for r in range(TPC):
        batch_t = order_all[r * NCORES : (r + 1) * NCORES]
        seq = range(NCORES) if r % 2 == 0 else range(NCORES - 1, -1, -1)
        for j, c in enumerate(seq):
            core_tiles[c].append(int(batch_t[j]) if j < len(batch_t) else -1)
    core_of_tile = np.full(n_tiles, -1, dtype=np.int64)
    slot_of_tile = np.full(n_tiles, -1, dtype=np.int64)
    for c in range(NCORES):
        for s, t in enumerate(core_tiles[c]):
            if t >= 0:
                core_of_tile[t] = c
                slot_of_tile[t] = s

    # unified block-major table layout (all 3 layers)
    blk = slot_of_tile[tile_of_node] // SPB
    table_row = (
        blk * (NCORES * SPB * P)
        + core_of_tile[tile_of_node] * (SPB * P)
        + (slot_of_tile[tile_of_node] % SPB) * P
        + (np.arange(N) % P)
    )

    order = np.argsort(dst_tile, kind="stable")
    src_sorted = src[order]
    dst_sorted = dst[order]
    sorted_tiles = dst_tile[order]
    tile_starts = np.searchsorted(sorted_tiles, np.arange(n_tiles))
    tile_ends = np.searchsorted(sorted_tiles, np.arange(n_tiles), side="right")

    cnts = np.zeros((NCORES, TPC), dtype=np.int64)
    for c in range(NCORES):
        for s in range(TPC):
            t = core_tiles[c][s]
            if t >= 0:
                cnts[c, s] = tile_ends[t] - tile_starts[t]
    chunks = np.maximum(1, -(-cnts.max(axis=0) // P))  # per-slot max over cores
    chunk_base = np.cumsum(np.concatenate([[0], chunks[:-1]])).astype(int)
    TCH = int(chunks.sum())

    BLKROWS = NCORES * SPB * P
    idx32 = np.zeros((NCORES, P, TCH), dtype=np.int32)
    dstin_T = np.full((NCORES, P, TCH), -1.0, dtype=np.float32)
    # per-chunk max table row (over cores) -> which AG blocks a gather needs;
    # edges are sorted by table row within a slot so early chunks only
    # depend on early AllGather blocks.
    chunk_hiblk = np.ones(TCH, dtype=np.int64)
    for c in range(NCORES):
        for s in range(TPC):
            t = core_tiles[c][s]
            if t < 0:
                continue
            lo, hi = tile_starts[t], tile_ends[t]
            rows = table_row[src_sorted[lo:hi]]
            din = dst_sorted[lo:hi] % P
            o = np.argsort(rows, kind="stable")
            rows = rows[o]
            din = din[o]
            nch = int(chunks[s])
            L = nch * P
            a = np.zeros(L, dtype=np.int32)
            a[: len(rows)] = rows
            d = np.full(L, -1.0, dtype=np.float32)
            d[: len(din)] = din
            cb = chunk_base[s]
            idx32[c, :, cb : cb + nch] = a.reshape(nch, P).T
            dstin_T[c, :, cb : cb + nch] = d.reshape(nch, P).T
            hb = -(-(a.reshape(nch, P).max(axis=1) + 1) // BLKROWS)
            chunk_hiblk[cb : cb + nch] = np.maximum(
                chunk_hiblk[cb : cb + nch], np.maximum(hb, 1)
            )

    dinv_slot = np.zeros((NCORES, P, TPC), dtype=np.float32)
    rdinv_row = np.zeros((NCORES, 1, SLOT_ROWS), dtype=np.float32)
    gidvec = np.full((NCORES, SLOT_ROWS), -1, dtype=np.int64)
    node_perm = np.full((NCORES, SLOT_ROWS), -1, dtype=np.int64)
    batch = np.asarray(batch, dtype=np.int64)
    for c in range(NCORES):
        for s in range(TPC):
            t = core_tiles[c][s]
            if t < 0:
                continue
            v0 = t * P
            v1 = min(v0 + P, N)
            n = v1 - v0
            dinv_slot[c, :n, s] = dinv[v0:v1]
            rdinv_row[c, 0, s * P : s * P + n] = rdinv[v0:v1]
            gidvec[c, s * P : s * P + n] = batch[v0:v1]
            node_perm[c, s * P : s * P + n] = np.arange(v0, v1)

    ohg = (gidvec[:, None, :] == np.arange(G)[None, :, None]).astype(np.float32)

    return dict(
        chunks=chunks,
        TCH=TCH,
        chunk_hiblk=chunk_hiblk,
        idx32=idx32,
        dstin_T=dstin_T,
        dinv_slot=dinv_slot,
        rdinv_row=rdinv_row,
        ohg=ohg,
        node_perm=node_perm,
    )


# ------------------------------------------------------------- bass program
def build_program(chunks, chunk_hiblk):
    import concourse.bacc as bacc
    import concourse.bass as bass
    import concourse.tile as tile
    from concourse import mybir
    from concourse.masks import make_identity

    FDT = {
        "f32": mybir.dt.float32,
        "f32r": mybir.dt.float32r,
        "bf16": mybir.dt.bfloat16,
    }[AGG_DT]
    F32 = mybir.dt.float32
    I32 = mybir.dt.int32
    BLKROWS = NCORES * SPB * P

    chunks = np.asarray(chunks)
    chunk_base = np.cumsum(np.concatenate([[0], chunks[:-1]])).astype(int)
    TCH = int(chunks.sum())

    nc = bacc.Bacc("TRN2", target_bir_lowering=False)
    dp = nc.declare_dram_parameter
    xT = dp("xT", [P, SLOT_ROWS], FDT, isOutput=False)
    idx_in = dp("idx32", [P, TCH], I32, isOutput=False)
    dstin = dp("dstin", [P, TCH], FDT, isOutput=False)
    iota_in = dp("iota_in", [P, P], FDT, isOutput=False)
    dinv_in = dp("dinv_in", [P, TPC], F32, isOutput=False)
    rdinv_in = dp("rdinv_in", [1, SLOT_ROWS], F32, isOutput=False)
    brow_in = dp("brow_in", [1, 3 * P], F32, isOutput=False)
    ones_in = dp("ones_in", [1, P], F32, isOutput=False)
    W_in = [dp(f"W{i}", [P, P], FDT, isOutput=False) for i in range(3)]
    fc1a_in = dp("fc1a", [P, P], FDT, isOutput=False)
    fc2w_in = dp("fc2w", [P, OUTC], FDT, isOutput=False)
    fc2brow_in = dp("fc2brow", [1, OUTC], F32, isOutput=False)
    ohg_in = dp("ohg", [G, SLOT_ROWS], FDT, isOutput=False)
    qeT_in = dp("qeT", [QD, G], F32, isOutput=False)
    fc0w_in = dp("fc0w", [QD, P], F32, isOutput=False)
    fc0bb_in = dp("fc0bb", [P, P], F32, isOutput=False)
    fc1b_in = dp("fc1b", [P, P], F32, isOutput=False)
    fc1bb_in = dp("fc1bb", [P, P], F32, isOutput=False)
    out_d = dp("out", [SLOT_ROWS, OUTC], F32, isOutput=True)

    cc_in = nc.dram_tensor("cc_in", [SLOT_ROWS, P], FDT)
    tables = [
        nc.dram_tensor(f"table{l}", [NT, P], FDT, addr_space="Shared")
        for l in range(3)
    ]

    with tile.TileContext(nc) as tc, ExitStack() as ctx:
        const = ctx.enter_context(tc.tile_pool(name="const", bufs=1))
        gp = ctx.enter_context(tc.tile_pool(name="gp", bufs=24))
        ohp = ctx.enter_context(tc.tile_pool(name="ohp", bufs=3))
        psp = ctx.enter_context(tc.tile_pool(name="psp", bufs=3, space="PSUM"))
        psagg = ctx.enter_context(tc.tile_pool(name="psagg", bufs=3, space="PSUM"))
        pst = ctx.enter_context(tc.tile_pool(name="pst", bufs=2, space="PSUM"))
        epi = ctx.enter_context(tc.tile_pool(name="epi", bufs=3))

        # ---- constants (xT + W0 first: they gate layer-0 production,
        # which gates the startup AllGathers)
        xT_sb = const.tile([P, SLOT_ROWS], FDT)
        nc.sync.dma_start(out=xT_sb[:], in_=xT[:])
        W_sb = []
        for i in range(3):
            w = const.tile([P, P], FDT, tag=f"W{i}")
            nc.sync.dma_start(out=w[:], in_=W_in[i][:])
            W_sb.append(w)
        dinv_sb = const.tile([P, TPC], F32)
        nc.sync.dma_start(out=dinv_sb[:], in_=dinv_in[:])
        iota_sb = const.tile([P, P], FDT)
        nc.sync.dma_start(out=iota_sb[:], in_=iota_in[:])
        idx_sb = const.tile([P, TCH], I32)
        nc.scalar.dma_start(out=idx_sb[:], in_=idx_in[:])
        dstin_sb = const.tile([P, TCH], FDT)
        nc.scalar.dma_start(out=dstin_sb[:], in_=dstin[:])
        rdinv_sb = const.tile([1, SLOT_ROWS], F32)
        nc.sync.dma_start(out=rdinv_sb[:], in_=rdinv_in[:])
        brow_sb = const.tile([1, 3 * P], F32)
        nc.sync.dma_start(out=brow_sb[:], in_=brow_in[:])
        ones_sb = const.tile([1, P], F32)
        nc.sync.dma_start(out=ones_sb[:], in_=ones_in[:])
        fc1a_sb = const.tile([P, P], FDT)
        nc.sync.dma_start(out=fc1a_sb[:], in_=fc1a_in[:])
        fc2w_sb = const.tile([P, OUTC], FDT)
        nc.sync.dma_start(out=fc2w_sb[:], in_=fc2w_in[:])
        fc2brow_sb = const.tile([1, OUTC], F32)
        nc.sync.dma_start(out=fc2brow_sb[:], in_=fc2brow_in[:])
        ohg_sb = const.tile([G, SLOT_ROWS], FDT)
        nc.sync.dma_start(out=ohg_sb[:], in_=ohg_in[:])
        ident = const.tile([P, P], F32)
        make_identity(nc, ident[:])
        ident_r = const.tile([P, P], FDT, tag="ident_r")
        nc.vector.tensor_copy(out=ident_r[:], in_=ident[:])

        # ---- question path: qq = relu(qe@fc0+fc0_b)@fc1b + fc1_b (on-chip)
        qe_sb = const.tile([P, 6 * G], F32)
        fc0w_sb = const.tile([P, 6 * P], F32)
        for k in range(6):
            nc.sync.dma_start(
                out=qe_sb[:, k * G : (k + 1) * G], in_=qeT_in[k * P : (k + 1) * P, :]
            )
            nc.sync.dma_start(
                out=fc0w_sb[:, k * P : (k + 1) * P],
                in_=fc0w_in[k * P : (k + 1) * P, :],
            )
        fc0bb_sb = const.tile([P, P], F32)
        nc.sync.dma_start(out=fc0bb_sb[:], in_=fc0bb_in[:])
        fc1b_sb = const.tile([P, P], F32)
        nc.sync.dma_start(out=fc1b_sb[:], in_=fc1b_in[:])
        fc1bb_sb = const.tile([P, P], F32)
        nc.sync.dma_start(out=fc1bb_sb[:], in_=fc1bb_in[:])

        pq = psp.tile([G, P], F32, space="PSUM", tag="mm")
        for k in range(6):
            nc.tensor.matmul(
                out=pq[:],
                lhsT=qe_sb[:, k * G : (k + 1) * G],
                rhs=fc0w_sb[:, k * P : (k + 1) * P],
                start=(k == 0),
                stop=(k == 5),
            )
        qtmp = epi.tile([G, P], F32, tag="qtmp")
        nc.vector.tensor_tensor(
            out=qtmp[:], in0=pq[:], in1=fc0bb_sb[:G, :], op=mybir.AluOpType.add
        )
        qrelu = epi.tile([G, P], F32, tag="qrelu")
        nc.scalar.activation(
            out=qrelu[:], in_=qtmp[:], func=mybir.ActivationFunctionType.Relu
        )
        pqt = psp.tile([P, G], F32, space="PSUM", tag="mm")
        nc.tensor.transpose(out=pqt[:], in_=qrelu[:], identity=ident[:G, :G])
        qT = epi.tile([P, G], F32, tag="qT")
        nc.scalar.copy(out=qT[:], in_=pqt[:])
        pqq = psp.tile([G, P], F32, space="PSUM", tag="mm")
        nc.tensor.matmul(
            out=pqq[:], lhsT=qT[:], rhs=fc1b_sb[:], start=True, stop=True
        )
        qtmp2 = epi.tile([G, P], F32, tag="qtmp2")
        nc.vector.tensor_tensor(
            out=qtmp2[:], in0=pqq[:], in1=fc1bb_sb[:G, :], op=mybir.AluOpType.add
        )
        qq_bf = const.tile([G, P], FDT, tag="qq_bf")
        nc.vector.tensor_copy(out=qq_bf[:], in_=qtmp2[:])

        # resident own-slice h~ buffers (self-loop term source), layer parity
        hs_keep = [
            const.tile([P, SLOT_ROWS], FDT, tag=f"hsk{i}", name=f"hsk{i}")
            for i in range(2)
        ]

        def allgather_block(l, j):
            r0 = j * SPB * P
            r1 = (j + 1) * SPB * P
            nc.gpsimd.collective_compute(
                "AllGather",
                mybir.AluOpType.bypass,
                replica_groups=[list(range(NCORES))],
                ins=[cc_in[r0:r1].opt()],
                outs=[
                    tables[l][
                        j * NCORES * SPB * P : (j + 1) * NCORES * SPB * P
                    ].opt()
                ],
            )

        # ---- layer 0 production: h~0 = dinv * (x @ W0), block AGs
        for s in range(TPC):
            pp = psp.tile([P, P], F32, space="PSUM", tag="mm")
            nc.tensor.matmul(
                out=pp[:],
                lhsT=xT_sb[:, s * P : (s + 1) * P],
                rhs=W_sb[0][:],
                start=True,
                stop=True,
            )
            hs = hs_keep[0][:, s * P : (s + 1) * P]
            nc.scalar.activation(
                out=hs,
                in_=pp[:],
                func=mybir.ActivationFunctionType.Copy,
                scale=dinv_sb[:, s : s + 1],
            )
            nc.sync.dma_start(out=cc_in[s * P : (s + 1) * P, :], in_=hs)
            if (s + 1) % SPB == 0:
                allgather_block(0, s // SPB)

        # ---- 3 aggregation layers
        for l in range(3):
            table = tables[l]
            for s in range(TPC):
                nch = int(chunks[s])
                cb = int(chunk_base[s])
                ps = psagg.tile([P, P], F32, space="PSUM", tag="agg")
                # one-hot for the whole slot in one DVE op
                oh = ohp.tile([P, nch * P], FDT, tag="oh")
                nc.vector.tensor_tensor(
                    out=oh[:].rearrange("p (k j) -> p k j", k=nch),
                    in0=dstin_sb[:, cb : cb + nch].to_broadcast([P, nch, P]),
                    in1=iota_sb[:]
                    .rearrange("p (one j) -> p one j", one=1)
                    .to_broadcast([P, nch, P]),
                    op=mybir.AluOpType.is_equal,
                )
                for k in range(nch):
                    hb = int(chunk_hiblk[cb + k])
                    g = gp.tile([P, P], FDT, tag="g")
                    nc.gpsimd.indirect_dma_start(
                        out=g[:],
                        out_offset=None,
                        in_=table[0 : hb * BLKROWS],
                        in_offset=bass.IndirectOffsetOnAxis(
                            ap=idx_sb[:, cb + k : cb + k + 1], axis=0
                        ),
                    )
                    nc.tensor.matmul(
                        out=ps[:],
                        lhsT=oh[:, k * P : (k + 1) * P],
                        rhs=g[:],
                        start=(k == 0),
                        stop=False,
                    )
                # self-loop term: ps += I @ hs_keep[l%2][:, slot]
                nc.tensor.matmul(
                    out=ps[:],
                    lhsT=ident_r[:],
                    rhs=hs_keep[l % 2][:, s * P : (s + 1) * P],
                    start=False,
                    stop=False,
                )
                # bias term: ps += (1/dinv)[dst] (x) b[f]  (rank-1)
                nc.tensor.matmul(
                    out=ps[:],
                    lhsT=rdinv_sb[0:1, s * P : (s + 1) * P],
                    rhs=brow_sb[0:1, l * P : (l + 1) * P],
                    start=False,
                    stop=True,
                )
                # fused epilogue: h = relu(dinv*psum)
                hrelu = epi.tile([P, P], FDT, tag="hrelu")
                nc.scalar.activation(
                    out=hrelu[:],
                    in_=ps[:],
                    func=mybir.ActivationFunctionType.Relu,
                    scale=dinv_sb[:, s : s + 1],
                )
                pt = pst.tile([P, P], FDT, space="PSUM", tag="pt")
                nc.tensor.transpose(out=pt[:], in_=hrelu[:], identity=ident_r[:])
                hT = epi.tile([P, P], FDT, tag="hT")
                nc.vector.tensor_copy(out=hT[:], in_=pt[:])
                if l < 2:
                    # produce next layer h~ and stage for allgather
                    pp2 = psp.tile([P, P], F32, space="PSUM", tag="mm")
                    nc.tensor.matmul(
                        out=pp2[:],
                        lhsT=hT[:],
                        rhs=W_sb[l + 1][:],
                        start=True,
                        stop=True,
                    )
                    hs2 = hs_keep[(l + 1) % 2][:, s * P : (s + 1) * P]
                    nc.scalar.activation(
                        out=hs2,
                        in_=pp2[:],
                        func=mybir.ActivationFunctionType.Copy,
                        scale=dinv_sb[:, s : s + 1],
                    )
                    nc.sync.dma_start(
                        out=cc_in[s * P : (s + 1) * P, :], in_=hs2
                    )
                    if (s + 1) % SPB == 0:
                        allgather_block(l + 1, s // SPB)
                else:
                    # MLP head: u = relu(h3@fc1a + qq[gid]); out = u@fc2+b
                    pm = psp.tile([P, P], F32, space="PSUM", tag="mm")
                    nc.tensor.matmul(
                        out=pm[:], lhsT=hT[:], rhs=fc1a_sb[:], start=True,
                        stop=False,
                    )
                    nc.tensor.matmul(
                        out=pm[:],
                        lhsT=ohg_sb[:, s * P : (s + 1) * P],
                        rhs=qq_bf[:],
                        start=False,
                        stop=True,
                    )
                    ur = epi.tile([P, P], FDT, tag="ur")
                    nc.scalar.activation(
                        out=ur[:], in_=pm[:],
                        func=mybir.ActivationFunctionType.Relu,
                    )
                    pt2 = pst.tile([P, P], FDT, space="PSUM", tag="pt")
                    nc.tensor.transpose(
                        out=pt2[:], in_=ur[:], identity=ident_r[:]
                    )
                    uT = epi.tile([P, P], FDT, tag="uT")
                    nc.vector.tensor_copy(out=uT[:], in_=pt2[:])
                    po = psp.tile([P, OUTC], F32, space="PSUM", tag="mm")
                    nc.tensor.matmul(
                        out=po[:], lhsT=uT[:], rhs=fc2w_sb[:], start=True,
                        stop=False,
                    )
                    nc.tensor.matmul(
                        out=po[:],
                        lhsT=ones_sb[0:1, :],
                        rhs=fc2brow_sb[0:1, :],
                        start=False,
                        stop=True,
                    )
                    ob = epi.tile([P, OUTC], F32, tag="ob")
                    nc.vector.tensor_copy(out=ob[:], in_=po[:])
                    nc.sync.dma_start(
                        out=out_d[s * P : (s + 1) * P, :], in_=ob[:]
                    )
    nc.compile()
    return nc


# ---------------------------------------------------------------- interface
_CACHE = {}


def kernel(**inputs):
    trace = bool(int(os.environ.get("GCN_TRACE", "0")))
    if trace:
        _install_axon_prof()
    from concourse.bass_utils import run_bass_kernel_spmd

    x = np.ascontiguousarray(np.asarray(inputs["x"], dtype=np.float32))
    qe = np.asarray(inputs["question_embedding"], dtype=np.float32)
    pp = preprocess(inputs["edge_index"], inputs["batch"])
    chunks = pp["chunks"]

    key = (tuple(chunks.tolist()), tuple(pp["chunk_hiblk"].tolist()))
    if key not in _CACHE:
        _CACHE[key] = build_program(chunks, pp["chunk_hiblk"])
    nc = _CACHE[key]

    fdt = np.dtype("bfloat16") if BF16 else np.float32
    W = [np.asarray(inputs[f"W{i}"], np.float32) for i in range(3)]
    b = [np.asarray(inputs[f"b{i}"], np.float32) for i in range(3)]
    fc0_w = np.asarray(inputs["fc0_w"], np.float32)
    fc0_b = np.asarray(inputs["fc0_b"], np.float32)
    fc1_w = np.asarray(inputs["fc1_w"], np.float32)
    fc1_b = np.asarray(inputs["fc1_b"], np.float32)
    fc2_w = np.asarray(inputs["fc2_w"], np.float32)
    fc2_b = np.asarray(inputs["fc2_b"], np.float32)

    iota = np.broadcast_to(np.arange(P, dtype=np.float32), (P, P)).astype(fdt)
    common = {
        "iota_in": np.ascontiguousarray(iota),
        "W0": W[0].astype(fdt),
        "W1": W[1].astype(fdt),
        "W2": W[2].astype(fdt),
        "brow_in": np.concatenate(b).reshape(1, 3 * P).astype(np.float32),
        "ones_in": np.ones((1, P), np.float32),
        "qeT": np.ascontiguousarray(qe.T),
        "fc0w": fc0_w,
        "fc0bb": np.broadcast_to(fc0_b, (P, P)).copy(),
        "fc1a": np.ascontiguousarray(fc1_w[:P]).astype(fdt),
        "fc1b": np.ascontiguousarray(fc1_w[P:]),
        "fc1bb": np.broadcast_to(fc1_b, (P, P)).copy(),
        "fc2w": fc2_w.astype(fdt),
        "fc2brow": fc2_b.reshape(1, OUTC).astype(np.float32),
    }

    in_maps = []
    for c in range(NCORES):
        xTc = np.zeros((P, SLOT_ROWS), dtype=np.float32)
        valid = pp["node_perm"][c] >= 0
        xTc[:, valid] = x[pp["node_perm"][c][valid]].T
        m = dict(common)
        m["xT"] = np.ascontiguousarray(xTc.astype(fdt))
        m["idx32"] = np.ascontiguousarray(pp["idx32"][c])
        m["dstin"] = np.ascontiguousarray(pp["dstin_T"][c].astype(fdt))
        m["dinv_in"] = np.ascontiguousarray(pp["dinv_slot"][c])
        m["rdinv_in"] = np.ascontiguousarray(pp["rdinv_row"][c])
        m["ohg"] = np.ascontiguousarray(pp["ohg"][c].astype(fdt))
        in_maps.append(m)

    res = run_bass_kernel_spmd(
        nc,
        in_maps,
        list(range(NCORES)),
        trace=trace,
    )
    kernel.last_result = res

    out = np.zeros((N, OUTC), dtype=np.float32)
    for c in range(NCORES):
        valid = pp["node_perm"][c] >= 0
        out[pp["node_perm"][c][valid]] = res.results[c]["out"][valid]
    return out
